# revision 1
# baseline (speedup 1.0000x reference)
"""Trainium2 Bass kernel for nn_BinaryMemory (retrieval_knn).

reference:
    gated = sigmoid(query @ W.T + b)                      # [1, D], D=4096
    sims  = 1 - mean(|memory - gated|, axis=-1)           # [N],   N=16384
    mask  = sims >= 0.8

Sharding (8 cores, no collectives): shard the D axis. Core c owns
d-chunk [c*512, (c+1)*512):
  - W rows c*512..c*512+511  -> computes gated[c*512:(c+1)*512] locally
    (dot products via scalar_tensor_tensor with f32 sum-accumulate on DVE)
  - memory[:, c*512:(c+1)*512] -> partial L1 sums over its d-chunk for
    all 16384 rows
  - outputs partial sums [128, 128] f32; host reindexes, sums the 8
    cores' partials and applies sims = 1 - s/D, mask = sims >= 0.8.

W / query / memory stream in as fp16 (host-side cast): halves the HBM
traffic of this memory-bound kernel and gives the DVE 16-bit 2x mode on
the hot subtract. All reductions accumulate in f32; quantization error
on sims is ~5e-6 relative (f32 build measures 1.7e-7).

Per-tile pipeline: DVE subtract + 8x ScalarE Abs-with-accumulate, with
6 of 16 tiles handled entirely on DVE via the fused abs-reduce so the
two engines finish together. The gated row is broadcast to partitions
with PE row-select matmuls (no DMA on the gate critical path). All bulk
DMAs ride the sync HWDGE ring: one ring sustains ~309 GB/s (vs 247 mixed
with SWDGE) and its per-engine FIFO guarantees the gate weights land
before the mem stream starts competing for HBM.

Memory tile t holds rows t*1024..t*1024+1023; partition p holds the 8
consecutive rows t*1024+p*8+j (8 KB contiguous DMA runs). Per-core HBM
traffic ~21 MB.
"""
import sys

sys.path.insert(0, "/opt/trn_rl_repo")

import numpy as np

import concourse.bacc as bacc
import concourse.mybir as mybir
import concourse.tile as tile
from concourse.bass_utils import run_bass_kernel_spmd

N_CORES = 8
D = 4096
N = 16384
D_SH = D // N_CORES          # 512 dims per core
W_TILES = D_SH // 128        # 4 gate-weight tiles [128, 4096]
GP = 8                       # row-groups packed per memory tile
M_TILES = N // (128 * GP)    # 16 memory tiles [128, 8*512]
THRESHOLD = 0.8
A_TILES = {2, 5, 8, 11, 13}       # DVE-only abs-reduce tiles
H_TILES = {14, 15}                # tail tiles: reduce+ABS split across engines

_CACHE = {}


def _build():
    f32 = mybir.dt.float32
    f16 = mybir.dt.float16
    nc = bacc.Bacc(
        "TRN2", target_bir_lowering=False, debug=False, num_devices=N_CORES
    )

    qb = nc.dram_tensor("qb", [128, D], f16, kind="ExternalInput")
    w = nc.dram_tensor("w", [D_SH, D], f16, kind="ExternalInput")
    b = nc.dram_tensor("b", [D_SH], f32, kind="ExternalInput")
    mem = nc.dram_tensor("mem", [N, D_SH], f16, kind="ExternalInput")
    ident = nc.dram_tensor("ident", [128, 128], f32, kind="ExternalInput")
    # sel[k, wt*128+m] = 1 iff k==wt (row-select stationaries)
    sel = nc.dram_tensor(
        "sel", [W_TILES, W_TILES * 128], f16, kind="ExternalInput"
    )
    partials = nc.dram_tensor(
        "partials", [128, M_TILES * GP], f32, kind="ExternalOutput"
    )

    with tile.TileContext(nc) as tc:
        with (
            tc.tile_pool(name="const", bufs=1) as cpool,
            tc.tile_pool(name="big", bufs=9) as bpool,
            tc.tile_pool(name="diff", bufs=4) as dpool,
            tc.tile_pool(name="absout", bufs=2) as apool,
            tc.tile_pool(name="small", bufs=1) as spool,
            tc.tile_pool(name="psum", bufs=1, space="PSUM") as ppool,
        ):
            # The scalar-engine HWDGE ring carries ONLY gate traffic: a
            # gate-dependent DMA on the sync/gpsimd rings would block the
            # FIFO mem stream behind the gate.
            q_b = dpool.tile([128, D], f16, tag="diff")
            nc.scalar.dma_start(out=q_b[:], in_=qb[:])
            id_sb = cpool.tile([128, 128], f32, tag="ident")
            nc.scalar.dma_start(out=id_sb[:], in_=ident[:])
            sel_sb = spool.tile([W_TILES, W_TILES * 128], f16, tag="sel")
            nc.scalar.dma_start(out=sel_sb[:], in_=sel[:])
            b_row = spool.tile([W_TILES, 128], f32, tag="brow")
            nc.scalar.dma_start(
                out=b_row[:], in_=b[:].rearrange("(t p) -> t p", p=128)
            )

            # ---- gate: z[j] = sum_d W[j, d] * q[d], j = wt*128 + p ----
            z_col = spool.tile([128, W_TILES], f32, tag="zcol")
            for wt in range(W_TILES):
                w_tile = bpool.tile([128, D], f16, tag="m")
                nc.sync.dma_start(
                    out=w_tile[:], in_=w[wt * 128 : (wt + 1) * 128, :]
                )
                # scalar_tensor_tensor has no 16-bit 2x uop (measured
                # 4.34 us); TT mult (2x, 2.2 us) + ScalarE Copy-accumulate
                # gets the dot product off the critical path sooner
                prod = dpool.tile([128, D], f16, tag="diff")
                nc.vector.tensor_mul(prod[:], w_tile[:], q_b[:])
                gacc = apool.tile([128, D], f16, tag="gacc")
                nc.scalar.activation(
                    gacc[:],
                    prod[:],
                    mybir.ActivationFunctionType.Copy,
                    accum_out=z_col[:, wt : wt + 1],
                )

            # transpose z to row layout [wt, p]; add b, sigmoid there.
            # The little z transpose parks in a corner of the g PSUM tile
            # (Tile orders the later overwrite after the reads).
            g_ps = ppool.tile([128, D_SH], f32, tag="gps")
            z_ps = g_ps[0:W_TILES, 0:128]
            nc.tensor.transpose(z_ps, z_col[:], id_sb[:])
            zb_row = spool.tile([W_TILES, 128], f32, tag="zbrow")
            nc.vector.tensor_add(zb_row[:], z_ps, b_row[:])
            g_row = spool.tile([W_TILES, 128], f16, tag="grow")
            nc.scalar.activation(
                g_row[:], zb_row[:], mybir.ActivationFunctionType.Sigmoid
            )
            # broadcast g straight from g_row [4,128]: matmul with the
            # row-select stationary sel_wt gives out[p, n] = g_row[wt, n]
            # for every partition p -- no DMA in the chain.
            for wt in range(W_TILES):
                nc.tensor.matmul(
                    g_ps[:, wt * 128 : (wt + 1) * 128],
                    sel_sb[:, wt * 128 : (wt + 1) * 128],
                    g_row[:],
                )
            # materialize the replicated gate row in fp16 (plain 2D APs
            # measure faster than step-0 broadcast APs on the hot subtract)
            g_rep = cpool.tile([128, GP * D_SH], f16, tag="grep")
            nc.vector.tensor_copy(g_rep[:, 0:D_SH], g_ps[:])
            for j in range(1, GP):
                nc.vector.tensor_copy(
                    g_rep[:, j * D_SH : (j + 1) * D_SH], g_rep[:, 0:D_SH]
                )

            # ---- sims partials ----
            # tile t: partition p, free (j, d) = mem[t*1024 + p*8 + j, d]
            memv = mem[:].rearrange("(t p j) d -> t p j d", p=128, j=GP)
            sums = spool.tile([128, M_TILES * GP], f32, tag="sums")
            for t in range(M_TILES):
                m_tile = bpool.tile([128, GP * D_SH], f16, tag="m")
                nc.sync.dma_start(
                    out=m_tile[:].rearrange("p (j d) -> p j d", j=GP),
                    in_=memv[t],
                )
                diff = dpool.tile([128, GP * D_SH], f16, tag="diff")
                nc.vector.tensor_sub(diff[:], m_tile[:], g_rep[:])
                if t in A_TILES:
                    nc.vector.tensor_reduce(
                        out=sums[:, t * GP : (t + 1) * GP],
                        in_=diff[:].rearrange("p (j d) -> p j d", j=GP),
                        axis=mybir.AxisListType.X,
                        op=mybir.AluOpType.add,
                        apply_absolute_value=True,
                    )
                elif t in H_TILES:
                    # tail: half the groups on each engine -> ~3 us drain
                    half = GP // 2
                    nc.vector.tensor_reduce(
                        out=sums[:, t * GP : t * GP + half],
                        in_=diff[:, 0 : half * D_SH].rearrange(
                            "p (j d) -> p j d", j=half
                        ),
                        axis=mybir.AxisListType.X,
                        op=mybir.AluOpType.add,
                        apply_absolute_value=True,
                    )
                    for j in range(half, GP):
                        a_out = apool.tile([128, D_SH], f16, tag="absout")
                        col = t * GP + j
                        nc.scalar.activation(
                            a_out[:],
                            diff[:, j * D_SH : (j + 1) * D_SH],
                            mybir.ActivationFunctionType.Abs,
                            accum_out=sums[:, col : col + 1],
                        )
                else:
                    for j in range(GP):
                        a_out = apool.tile([128, D_SH], f16, tag="absout")
                        col = t * GP + j
                        nc.scalar.activation(
                            a_out[:],
                            diff[:, j * D_SH : (j + 1) * D_SH],
                            mybir.ActivationFunctionType.Abs,
                            accum_out=sums[:, col : col + 1],
                        )

            nc.sync.dma_start(out=partials[:], in_=sums[:])

    nc.compile()
    return nc


def _get_nc():
    if "nc" not in _CACHE:
        _CACHE["nc"] = _build()
    return _CACHE["nc"]


def make_aux_inputs():
    ident = np.eye(128, dtype=np.float32)
    sel = np.zeros((W_TILES, W_TILES * 128), dtype=np.float16)
    for wt in range(W_TILES):
        sel[wt, wt * 128 : (wt + 1) * 128] = 1.0
    return ident, sel


def kernel(query, W, b, memory, _trace=False, _return_raw=False):
    query = np.asarray(query, dtype=np.float32)
    W = np.asarray(W, dtype=np.float32)
    b = np.asarray(b, dtype=np.float32)
    memory = np.asarray(memory, dtype=np.float32)
    ident, sel = make_aux_inputs()
    q_bcast = np.ascontiguousarray(
        np.broadcast_to(query.reshape(1, D).astype(np.float16), (128, D))
    )
    W16 = W.astype(np.float16)
    mem16 = memory.astype(np.float16)

    in_maps = []
    for c in range(N_CORES):
        sl = slice(c * D_SH, (c + 1) * D_SH)
        in_maps.append(
            {
                "qb": q_bcast,
                "w": np.ascontiguousarray(W16[sl, :]),
                "b": np.ascontiguousarray(b[sl]),
                "mem": np.ascontiguousarray(mem16[:, sl]),
                "ident": ident,
                "sel": sel,
            }
        )

    nc = _get_nc()
    res = run_bass_kernel_spmd(
        nc, in_maps, list(range(N_CORES)), trace=_trace
    )

    total = np.zeros(N, dtype=np.float64)
    for c in range(N_CORES):
        mat = res.results[c]["partials"]  # [128 (p), 128 (t*8+j)]
        # row n = t*1024 + p*8 + j
        part = mat.reshape(128, M_TILES, GP).transpose(1, 0, 2).reshape(N)
        total += part.astype(np.float64)
    sims = (1.0 - total / D).astype(np.float32)
    mask = sims >= THRESHOLD
    if _return_raw:
        return (sims, mask), res
    return sims, mask



# revision 5
# speedup vs baseline: 1.2628x; 1.2628x over previous
"""Trainium2 Bass kernel for nn_BinaryMemory (retrieval_knn).

reference:
    gated = sigmoid(query @ W.T + b)                      # [1, D], D=4096
    sims  = 1 - mean(|memory - gated|, axis=-1)           # [N],   N=16384
    mask  = sims >= 0.8

Sharding (8 cores, no collectives): shard the D axis; core c owns
d-chunk [c*512, (c+1)*512). All bulk tensors stream as fp8_e3m4 (1 byte,
4 mantissa bits; every operand lives in (0,1) or N(0,1) so range is a
non-issue and the quantization noise on sims is ~1e-3 relative, versus
a 2e-2 budget). Per-core HBM traffic ~10.1 MB vs 21 MB for the fp16
variant; one HWDGE ring sustains ~320 GB/s so DMA is the ~33 us floor.

Layout is d-on-partitions (memory shard transposed host-side to
[512 d, 16384 n]): the gate value g[d] becomes a per-partition scalar,
which unlocks one-pass |m - g| on BOTH compute engines:
  - ScalarE: Abs activation with per-partition bias = -g[d] (1 elem/cyc).
  - DVE: tensor_scalar subtract (per-partition scalar g, ~0.27 cyc/elem
    at fp8) + sign-bit clear via bitwise_and 0x7F on a uint8 bitcast
    (~0.3 cyc/elem). Fused sub+abs isn't expressible in one stock op
    (ISA rejects arith op0 + bitwise op1, and accum_out forces 1x), so
    two cheap fp8 passes beat everything else measured.
The n-axis reduction runs entirely on the idle PE: ones-vector
stationary [128,1], |diff| moving [128,512] -> psum row = column sums.
Each psum bank holds 4 group-rows at quadrant offsets {0,32,64,96}
(tile_position), accumulated over the 4 d-chunks; one [128,512] f32
psum->SBUF copy drains 4 groups and a partition-strided DMA writes the
4 rows to DRAM.

The gate is also PE-only: host passes W-shard transposed [4096, 512]
fp8; q as 32 stationary columns [128,1]; 32 accumulating matmuls give
z = q @ W_sh.T as a psum row [1,512]; +b and sigmoid on [1,512]; a
SBUF->SBUF DMA scatters the row to [4,128]; PE-transpose yields g as
per-partition columns [128,4] for the scalar/bias slots.
"""
import sys

sys.path.insert(0, "/opt/trn_rl_repo")

import numpy as np
import ml_dtypes

import concourse.bacc as bacc
import concourse.mybir as mybir
import concourse.tile as tile
from concourse.bass_utils import run_bass_kernel_spmd

N_CORES = 8
D = 4096
N = 16384
D_SH = D // N_CORES            # 512 dims per core
DC = D_SH // 128               # 4 d-chunks (partition blocks)
NT = 4096                      # n per tile
NK = N // NT                   # 4 n-chunks
NG = NT // 512                 # 8 psum groups per tile
THRESHOLD = 0.8
# tiles (k, c) computed on ScalarE; the rest on DVE (2:1 rate ratio)
ACT_TILES = {(0, 1), (1, 0), (1, 3), (2, 3), (3, 2), (3, 3)}

_CACHE = {}


def _build():
    f32 = mybir.dt.float32
    f8 = mybir.dt.float8e3
    u8 = mybir.dt.uint8
    A = mybir.AluOpType
    nc = bacc.Bacc(
        "TRN2", target_bir_lowering=False, debug=False, num_devices=N_CORES
    )

    memT = nc.dram_tensor("memT", [D_SH, N], f8, kind="ExternalInput")
    wt = nc.dram_tensor("wt", [D, D_SH], f8, kind="ExternalInput")
    qcol = nc.dram_tensor("qcol", [128, 32], f8, kind="ExternalInput")
    brow = nc.dram_tensor("brow", [1, D_SH], f32, kind="ExternalInput")
    ones8 = nc.dram_tensor("ones8", [128, 1], f8, kind="ExternalInput")
    ident = nc.dram_tensor("ident", [128, 128], f32, kind="ExternalInput")
    outp = nc.dram_tensor("outp", [4 * NK * 2, 512], f32, kind="ExternalOutput")

    with tile.TileContext(nc) as tc:
        with (
            tc.tile_pool(name="const", bufs=1) as cpool,
            tc.tile_pool(name="mem", bufs=8) as mpool,
            tc.tile_pool(name="acts", bufs=2) as apool,
            tc.tile_pool(name="cp", bufs=4) as cppool,
            tc.tile_pool(name="small", bufs=1) as spool,
            tc.tile_pool(name="psg", bufs=1, space="PSUM") as ppg,
            tc.tile_pool(name="psm", bufs=6, space="PSUM") as ppm,
        ):
            # gate weights first on the sync ring (mem stream queues behind),
            # small gate tensors on the scalar ring.
            wt_sb = cpool.tile([128, 32 * D_SH], f8, tag="wt")
            nc.sync.dma_start(
                out=wt_sb[:].rearrange("p (j n) -> p j n", j=32),
                in_=wt[:].rearrange("(j p) n -> p j n", p=128),
            )
            qc_sb = spool.tile([128, 32], f8, tag="qc")
            nc.scalar.dma_start(out=qc_sb[:], in_=qcol[:])
            b_sb = spool.tile([1, D_SH], f32, tag="b")
            nc.scalar.dma_start(out=b_sb[:], in_=brow[:])
            ones_sb = spool.tile([128, 1], f8, tag="ones")
            nc.scalar.dma_start(out=ones_sb[:], in_=ones8[:])
            id_sb = cpool.tile([128, 128], f32, tag="id")
            nc.scalar.dma_start(out=id_sb[:], in_=ident[:])

            # ---- gate: z[1, 512] = sum_j qcol[:, j]^T @ wt chunk j ----
            zps = ppg.tile([1, D_SH], f32, tag="z")
            for j in range(32):
                nc.tensor.matmul(
                    zps[:],
                    qc_sb[:, j : j + 1],
                    wt_sb[:, j * D_SH : (j + 1) * D_SH],
                    start=(j == 0),
                    stop=(j == 31),
                    skip_group_check=True,
                )
            zrow = spool.tile([1, D_SH], f32, tag="zrow")
            nc.vector.tensor_tensor(zrow[:], zps[:], b_sb[:], A.add)
            grow = spool.tile([1, D_SH], f32, tag="grow")
            nc.scalar.activation(
                grow[:], zrow[:], mybir.ActivationFunctionType.Sigmoid
            )
            # scatter the row to 4 partitions, then PE-transpose to columns
            g4 = spool.tile([4, 128], f32, tag="g4")
            nc.scalar.dma_start(
                out=g4[:], in_=grow[:].rearrange("p (a b) -> p a b", a=4)
            )
            tps = ppg.tile([128, DC], f32, tag="tp")
            nc.tensor.transpose(tps[:], g4[:], id_sb[0:4, 0:4])
            gpos = spool.tile([128, DC], f32, tag="gpos")
            nc.vector.tensor_copy(gpos[:], tps[:])
            negg = spool.tile([128, DC], f32, tag="negg")
            nc.vector.tensor_scalar(negg[:], tps[:], -1.0, None, A.mult)

            # ---- main loop: |m - g| then PE column-sum into psum rows ----
            for k in range(NK):
                bank0 = ppm.tile([128, 512], f32, tag="bank")
                bank1 = ppm.tile([128, 512], f32, tag="bank")
                banks = [bank0, bank1]
                for c in range(DC):
                    mt = mpool.tile([128, NT], f8, tag="m")
                    nc.sync.dma_start(
                        out=mt[:],
                        in_=memT[c * 128 : (c + 1) * 128, k * NT : (k + 1) * NT],
                    )
                    if (k, c) in ACT_TILES:
                        at = apool.tile([128, NT], f8, tag="a")
                        nc.scalar.activation(
                            at[:],
                            mt[:],
                            mybir.ActivationFunctionType.Abs,
                            bias=negg[:, c : c + 1],
                        )
                        src = at
                    else:
                        nc.vector.tensor_scalar(
                            mt[:], mt[:], gpos[:, c : c + 1], None, A.subtract
                        )
                        nc.vector.tensor_scalar(
                            mt[:].bitcast(u8),
                            mt[:].bitcast(u8),
                            0x7F,
                            None,
                            A.bitwise_and,
                        )
                        src = mt
                    for j in range(NG):
                        nc.tensor.matmul(
                            banks[j // 4][32 * (j % 4) : 32 * (j % 4) + 1, :],
                            ones_sb[:],
                            src[:, j * 512 : (j + 1) * 512],
                            start=(c == 0),
                            stop=(c == DC - 1),
                            tile_position=(0, 32 * (j % 4)),
                            skip_group_check=True,
                        )
                for h in range(2):
                    cp = cppool.tile([128, 512], f32, tag="cp")
                    if h == 0:
                        nc.vector.tensor_copy(cp[:], banks[h][:])
                    else:
                        nc.scalar.activation(
                            cp[:],
                            banks[h][:],
                            mybir.ActivationFunctionType.Copy,
                        )
                    nc.scalar.dma_start(
                        out=outp[8 * k + 4 * h : 8 * k + 4 * h + 4, :],
                        in_=cp[0:128:32, :],
                    )

    nc.compile()
    return nc


def _get_nc():
    if "nc" not in _CACHE:
        _CACHE["nc"] = _build()
    return _CACHE["nc"]


def kernel(query, W, b, memory, _trace=False, _return_raw=False):
    f8 = ml_dtypes.float8_e3m4
    query = np.asarray(query, dtype=np.float32)
    W = np.asarray(W, dtype=np.float32)
    b = np.asarray(b, dtype=np.float32)
    memory = np.asarray(memory, dtype=np.float32)

    mem8T = np.ascontiguousarray(memory.astype(f8).T)       # [D, N] fp8
    W8 = W.astype(f8)
    q8 = np.ascontiguousarray(query.reshape(32, 128).astype(f8).T)  # [128, 32]
    ones = np.ones((128, 1), dtype=f8)
    ident = np.eye(128, dtype=np.float32)

    in_maps = []
    for c in range(N_CORES):
        sl = slice(c * D_SH, (c + 1) * D_SH)
        in_maps.append(
            {
                "memT": np.ascontiguousarray(mem8T[sl, :]),
                "wt": np.ascontiguousarray(W8[sl, :].T),
                "qcol": q8,
                "brow": np.ascontiguousarray(b[sl].reshape(1, D_SH)),
                "ones8": ones,
                "ident": ident,
            }
        )

    nc = _get_nc()
    res = run_bass_kernel_spmd(
        nc, in_maps, list(range(N_CORES)), trace=_trace
    )

    total = np.zeros(N, dtype=np.float64)
    for c in range(N_CORES):
        total += res.results[c]["outp"].reshape(N).astype(np.float64)
    sims = (1.0 - total / D).astype(np.float32)
    mask = sims >= THRESHOLD
    if _return_raw:
        return (sims, mask), res
    return sims, mask


# revision 7
# speedup vs baseline: 1.8234x; 1.4439x over previous
"""Trainium2 Bass kernel for nn_BinaryMemory (retrieval_knn).

reference:
    gated = sigmoid(query @ W.T + b)                      # [1, D], D=4096
    sims  = 1 - mean(|memory - gated|, axis=-1)           # [N],   N=16384
    mask  = sims >= 0.8

Sharding (8 cores, no collectives): shard the D axis; core c owns
d-chunk [c*512, (c+1)*512). All bulk tensors stream as fp8_e3m4 (1 byte,
4 mantissa bits; every operand lives in (0,1) or N(0,1) so range is a
non-issue; quantization puts ~1e-2 relative on sims vs the 2e-2 budget).
Per-core HBM ~10.1 MB; one HWDGE ring sustains ~320 GB/s (a second ring
adds nothing - HBM/core limit), so DMA floors at ~33 us.

Layout is d-on-partitions (memory shard transposed host-side to
[512 d, 16384 n]) so the gate value g[d] is a per-partition scalar.
The elementwise work splits |m-g| = (m-g) - 2*min(m-g, 0):
  - DVE tiles: ONE stock pass, tensor_scalar(op0=subtract scalar1=g,
    op1=min scalar2=0) -> min-term (fp8 in/out runs the 2x port trick,
    ~2.1 us per [128,4096]; fused sub+abs is not expressible: the ISA
    rejects abs_max/bitwise op1 pairings and accum_out forces 1x).
  - ScalarE tiles: one Abs activation with per-partition bias -g (makes
    |m-g| directly, ~3.7 us/tile, dtype-agnostic).
The n-reduction runs on the idle PE: per 512-column group a psum row
accumulates ones^T @ m (issued as soon as the tile lands - no gate
dependency) and (-2*ones)^T @ minterm for DVE tiles, or ones^T @ |m-g|
for ScalarE tiles. The spurious +sum(g) from the m-term is cancelled on
the host using per-d-chunk gate sums that ride out through psum row 32.
Each psum bank holds 4 group-rows at quadrant offsets {0,32,64,96}
(tile_position); one [128,512] psum->SBUF copy drains 4 groups and a
partition-strided DMA writes the rows to DRAM.

The gate runs on the PE: host pre-packs W-shard^T as [128, 32*512] fp8
(contiguous 16 KB/partition DMA, split in 4 so matmuls start early);
q rides as 32 stationary columns; 32 accumulating matmuls produce
z = q @ W_sh.T as a psum row; +b (DVE) and sigmoid (ScalarE, activation
tables preloaded by dummy ops at t0); a SBUF->SBUF DMA scatters the row
to [4,128]; PE-transpose yields g as per-partition columns [128,4].
"""
import sys

sys.path.insert(0, "/opt/trn_rl_repo")

import numpy as np
import ml_dtypes

import concourse.bacc as bacc
import concourse.mybir as mybir
import concourse.tile as tile
from concourse.bass_utils import run_bass_kernel_spmd

N_CORES = 8
D = 4096
N = 16384
D_SH = D // N_CORES            # 512 dims per core
DC = D_SH // 128               # 4 d-chunks (partition blocks)
NT = 4096                      # n per tile
NK = N // NT                   # 4 n-chunks
NG = NT // 512                 # 8 psum groups per tile
THRESHOLD = 0.8
# tiles (k, c) computed on ScalarE via Abs; the rest on DVE via sub+min
ACT_TILES = {(0, 2), (1, 1), (1, 3), (2, 0), (3, 1), (3, 3)}
DVE_SETS = [
    [c for c in range(DC) if (k, c) not in ACT_TILES] for k in range(NK)
]

_CACHE = {}


def _build():
    f32 = mybir.dt.float32
    f8 = mybir.dt.float8e3
    A = mybir.AluOpType
    AF = mybir.ActivationFunctionType
    nc = bacc.Bacc(
        "TRN2", target_bir_lowering=False, debug=False, num_devices=N_CORES
    )

    memT = nc.dram_tensor("memT", [D_SH, N], f8, kind="ExternalInput")
    # W shard, host-packed to [128, 32*512]: partition p, chunk j holds
    # W.T[j*128 + p, :] (contiguous per-partition runs)
    wtp = nc.dram_tensor("wtp", [128, 32 * D_SH], f8, kind="ExternalInput")
    qcol = nc.dram_tensor("qcol", [128, 32], f8, kind="ExternalInput")
    brow = nc.dram_tensor("brow", [1, D_SH], f32, kind="ExternalInput")
    ones8 = nc.dram_tensor("ones8", [128, 1], f8, kind="ExternalInput")
    neg2 = nc.dram_tensor("neg2", [128, 1], f8, kind="ExternalInput")
    ones32 = nc.dram_tensor("ones32", [128, 1], f32, kind="ExternalInput")
    ident = nc.dram_tensor("ident", [4, 4], f32, kind="ExternalInput")
    outp = nc.dram_tensor("outp", [33, 512], f32, kind="ExternalOutput")

    with tile.TileContext(nc) as tc:
        with (
            tc.tile_pool(name="const", bufs=1) as cpool,
            tc.tile_pool(name="mem", bufs=8) as mpool,
            tc.tile_pool(name="dts", bufs=3) as dpool,
            tc.tile_pool(name="acts", bufs=2) as apool,
            tc.tile_pool(name="cp", bufs=4) as cppool,
            tc.tile_pool(name="small", bufs=1) as spool,
            tc.tile_pool(name="psg", bufs=1, space="PSUM") as ppg,
            tc.tile_pool(name="psm", bufs=6, space="PSUM") as ppm,
        ):
            # gate weights first on the sync ring (mem stream queues
            # behind); small tensors on the scalar ring.
            wt_sb = cpool.tile([128, 32 * D_SH], f8, tag="wt")
            for h in range(4):
                sl = slice(h * 8 * D_SH, (h + 1) * 8 * D_SH)
                nc.sync.dma_start(out=wt_sb[:, sl], in_=wtp[:, sl])
            qc_sb = spool.tile([128, 32], f8, tag="qc")
            nc.scalar.dma_start(out=qc_sb[:], in_=qcol[:])
            b_sb = spool.tile([1, D_SH], f32, tag="b")
            nc.scalar.dma_start(out=b_sb[:], in_=brow[:])
            ones_sb = spool.tile([128, 1], f8, tag="ones")
            nc.scalar.dma_start(out=ones_sb[:], in_=ones8[:])
            neg2_sb = spool.tile([128, 1], f8, tag="neg2")
            nc.scalar.dma_start(out=neg2_sb[:], in_=neg2[:])
            ones32_sb = spool.tile([128, 1], f32, tag="ones32")
            nc.scalar.dma_start(out=ones32_sb[:], in_=ones32[:])
            id_sb = spool.tile([4, 4], f32, tag="id")
            nc.scalar.dma_start(out=id_sb[:], in_=ident[:])
            # preload Sigmoid+Abs activation tables off the critical path
            dum = spool.tile([1, 16], f32, tag="dum")
            nc.scalar.activation(dum[:], b_sb[0:1, 0:16], AF.Sigmoid)
            nc.scalar.activation(dum[:], b_sb[0:1, 0:16], AF.Abs)

            # ---- gate: z[1, 512] = sum_j qcol[:, j]^T @ wt chunk j ----
            zps = ppg.tile([1, D_SH], f32, tag="z")
            for j in range(32):
                nc.tensor.matmul(
                    zps[:],
                    qc_sb[:, j : j + 1],
                    wt_sb[:, j * D_SH : (j + 1) * D_SH],
                    start=(j == 0),
                    stop=(j == 31),
                    skip_group_check=True,
                )
            zrow = spool.tile([1, D_SH], f32, tag="zrow")
            nc.vector.tensor_tensor(zrow[:], zps[:], b_sb[:], A.add)
            grow = spool.tile([1, D_SH], f32, tag="grow")
            nc.scalar.activation(grow[:], zrow[:], AF.Sigmoid)
            # scatter the row to 4 partitions, then PE-transpose to columns
            g4 = spool.tile([4, 128], f32, tag="g4")
            nc.scalar.dma_start(
                out=g4[:], in_=grow[:].rearrange("p (a b) -> p a b", a=4)
            )
            tps = ppg.tile([128, DC], f32, tag="tp")
            nc.tensor.transpose(tps[:], g4[:], id_sb[:])
            gpos = spool.tile([128, DC], f32, tag="gpos")
            nc.vector.tensor_copy(gpos[:], tps[:])
            negg = spool.tile([128, DC], f32, tag="negg")
            nc.vector.tensor_scalar(negg[:], tps[:], -1.0, None, A.mult)
            # per-d-chunk gate sums -> psum row (rides out via outp[32])
            gs = zps[0:1, 0:DC]
            nc.tensor.matmul(
                gs, ones32_sb[:], gpos[:], start=True, stop=True,
                skip_group_check=True,
            )
            gs_sb = spool.tile([1, DC], f32, tag="gs")
            nc.vector.tensor_copy(gs_sb[:], gs)
            nc.scalar.dma_start(out=outp[32:33, 0:DC], in_=gs_sb[:])

            # ---- main loop ----
            for k in range(NK):
                bank0 = ppm.tile([128, 512], f32, tag="bank")
                bank1 = ppm.tile([128, 512], f32, tag="bank")
                banks = [bank0, bank1]
                # per-group pass budget: DVE chunks cost 2 (m + min),
                # ACT chunks cost 1 (abs)
                total_passes = sum(
                    1 if (k, c) in ACT_TILES else 2 for c in range(DC)
                )
                seen = [0] * NG
                mts = []
                # phase 1: DMA + gate-independent m-term matmuls
                for c in range(DC):
                    mt = mpool.tile([128, NT], f8, tag="m")
                    nc.sync.dma_start(
                        out=mt[:],
                        in_=memT[c * 128 : (c + 1) * 128, k * NT : (k + 1) * NT],
                    )
                    mts.append(mt)
                    if (k, c) not in ACT_TILES:
                        for j in range(NG):
                            nc.tensor.matmul(
                                banks[j // 4][32 * (j % 4) : 32 * (j % 4) + 1, :],
                                ones_sb[:],
                                mt[:, j * 512 : (j + 1) * 512],
                                start=(seen[j] == 0),
                                stop=(seen[j] == total_passes - 1),
                                tile_position=(0, 32 * (j % 4)),
                                skip_group_check=True,
                            )
                            seen[j] += 1
                # phase 2: gate-dependent passes
                for c in range(DC):
                    mt = mts[c]
                    if (k, c) in ACT_TILES:
                        at = apool.tile([128, NT], f8, tag="a")
                        nc.scalar.activation(
                            at[:], mt[:], AF.Abs, bias=negg[:, c : c + 1]
                        )
                        src, stat = at, ones_sb
                    else:
                        dt = dpool.tile([128, NT], f8, tag="d")
                        nc.vector.tensor_scalar(
                            dt[:], mt[:], gpos[:, c : c + 1], 0.0,
                            A.subtract, A.min,
                        )
                        src, stat = dt, neg2_sb
                    for j in range(NG):
                        nc.tensor.matmul(
                            banks[j // 4][32 * (j % 4) : 32 * (j % 4) + 1, :],
                            stat[:],
                            src[:, j * 512 : (j + 1) * 512],
                            start=(seen[j] == 0),
                            stop=(seen[j] == total_passes - 1),
                            tile_position=(0, 32 * (j % 4)),
                            skip_group_check=True,
                        )
                        seen[j] += 1
                for h in range(2):
                    cp = cppool.tile([128, 512], f32, tag="cp")
                    if h == 0:
                        nc.vector.tensor_copy(cp[:], banks[h][:])
                    else:
                        nc.scalar.activation(cp[:], banks[h][:], AF.Copy)
                    nc.scalar.dma_start(
                        out=outp[8 * k + 4 * h : 8 * k + 4 * h + 4, :],
                        in_=cp[0:128:32, :],
                    )

    nc.compile()
    return nc


def _get_nc():
    if "nc" not in _CACHE:
        _CACHE["nc"] = _build()
    return _CACHE["nc"]


def kernel(query, W, b, memory, _trace=False, _return_raw=False):
    f8 = ml_dtypes.float8_e3m4
    query = np.asarray(query, dtype=np.float32)
    W = np.asarray(W, dtype=np.float32)
    b = np.asarray(b, dtype=np.float32)
    memory = np.asarray(memory, dtype=np.float32)

    mem8T = np.ascontiguousarray(memory.astype(f8).T)       # [D, N] fp8
    W8 = W.astype(f8)
    q8 = np.ascontiguousarray(query.reshape(32, 128).astype(f8).T)  # [128, 32]
    ones = np.ones((128, 1), dtype=f8)
    neg2 = np.full((128, 1), -2.0, dtype=f8)
    ones32 = np.ones((128, 1), dtype=np.float32)
    ident = np.eye(4, dtype=np.float32)

    in_maps = []
    for c in range(N_CORES):
        sl = slice(c * D_SH, (c + 1) * D_SH)
        # wtp[p, j*512 + n] = W.T[j*128 + p, n] = W8[sl][n, j*128+p]
        wsh = W8[sl, :]                       # [512, 4096]
        wtp = np.ascontiguousarray(
            wsh.T.reshape(32, 128, D_SH).transpose(1, 0, 2).reshape(128, -1)
        )
        in_maps.append(
            {
                "memT": np.ascontiguousarray(mem8T[sl, :]),
                "wtp": wtp,
                "qcol": q8,
                "brow": np.ascontiguousarray(b[sl].reshape(1, D_SH)),
                "ones8": ones,
                "neg2": neg2,
                "ones32": ones32,
                "ident": ident,
            }
        )

    nc = _get_nc()
    res = run_bass_kernel_spmd(
        nc, in_maps, list(range(N_CORES)), trace=_trace
    )

    total = np.zeros(N, dtype=np.float64)
    for c in range(N_CORES):
        out = res.results[c]["outp"]
        gsum = out[32, 0:DC].astype(np.float64)   # sum of g per d-chunk
        rows = out[0:32].reshape(NK, NG, 512)
        corr = np.array(
            [sum(gsum[ci] for ci in DVE_SETS[k]) for k in range(NK)]
        )
        total += (rows - corr[:, None, None]).reshape(N)
    sims = (1.0 - total / D).astype(np.float32)
    mask = sims >= THRESHOLD
    if _return_raw:
        return (sims, mask), res
    return sims, mask


# revision 8
# speedup vs baseline: 1.8533x; 1.0164x over previous
"""Trainium2 Bass kernel for nn_BinaryMemory (retrieval_knn).

reference:
    gated = sigmoid(query @ W.T + b)                      # [1, D], D=4096
    sims  = 1 - mean(|memory - gated|, axis=-1)           # [N],   N=16384
    mask  = sims >= 0.8

Sharding (8 cores, no collectives): shard the D axis; core c owns
d-chunk [c*512, (c+1)*512). All bulk tensors stream as fp8_e3m4 (1 byte,
4 mantissa bits; every operand lives in (0,1) or N(0,1) so range is a
non-issue; quantization puts ~1e-2 relative on sims vs the 2e-2 budget).
Per-core HBM ~10.1 MB; one HWDGE ring sustains ~320 GB/s (a second ring
adds nothing - HBM/core limit), so DMA floors at ~33 us.

Layout is d-on-partitions (memory shard transposed host-side to
[512 d, 16384 n]) so the gate value g[d] is a per-partition scalar.
The elementwise work splits |m-g| = (m-g) - 2*min(m-g, 0):
  - DVE tiles: ONE stock pass, tensor_scalar(op0=subtract scalar1=g,
    op1=min scalar2=0) -> min-term (fp8 in/out runs the 2x port trick,
    ~2.1 us per [128,4096]; fused sub+abs is not expressible: the ISA
    rejects abs_max/bitwise op1 pairings and accum_out forces 1x).
  - ScalarE tiles: one Abs activation with per-partition bias -g (makes
    |m-g| directly, ~3.7 us/tile, dtype-agnostic).
The n-reduction runs on the idle PE: per 512-column group a psum row
accumulates ones^T @ m (issued as soon as the tile lands - no gate
dependency) and (-2*ones)^T @ minterm for DVE tiles, or ones^T @ |m-g|
for ScalarE tiles. The spurious +sum(g) from the m-term is cancelled on
the host using per-d-chunk gate sums that ride out through psum row 32.
Each psum bank holds 4 group-rows at quadrant offsets {0,32,64,96}
(tile_position); one [128,512] psum->SBUF copy drains 4 groups and a
partition-strided DMA writes the rows to DRAM.

The gate runs on the PE: host pre-packs W-shard^T as [128, 32*512] fp8
(contiguous 16 KB/partition DMA, split in 4 so matmuls start early);
q rides as 32 stationary columns; 32 accumulating matmuls produce
z = q @ W_sh.T as a psum row; +b (DVE) and sigmoid (ScalarE, activation
tables preloaded by dummy ops at t0); a SBUF->SBUF DMA scatters the row
to [4,128]; PE-transpose yields g as per-partition columns [128,4].
"""
import sys

sys.path.insert(0, "/opt/trn_rl_repo")

import numpy as np
import ml_dtypes

import concourse.bacc as bacc
import concourse.mybir as mybir
import concourse.tile as tile
from concourse.bass_utils import run_bass_kernel_spmd

N_CORES = 8
D = 4096
N = 16384
D_SH = D // N_CORES            # 512 dims per core
DC = D_SH // 128               # 4 d-chunks (partition blocks)
NT = 4096                      # n per tile
NK = N // NT                   # 4 n-chunks
NG = NT // 512                 # 8 psum groups per tile
THRESHOLD = 0.8
# tiles (k, c) computed on ScalarE via Abs; the rest on DVE via sub+min
ACT_TILES = {(0, 2), (1, 1), (1, 3), (2, 0), (3, 1), (3, 3)}
DVE_SETS = [
    [c for c in range(DC) if (k, c) not in ACT_TILES] for k in range(NK)
]

_CACHE = {}


def _build():
    f32 = mybir.dt.float32
    f8 = mybir.dt.float8e3
    A = mybir.AluOpType
    AF = mybir.ActivationFunctionType
    nc = bacc.Bacc(
        "TRN2", target_bir_lowering=False, debug=False, num_devices=N_CORES
    )

    memT = nc.dram_tensor("memT", [D_SH, N], f8, kind="ExternalInput")
    # W shard, host-packed to [128, 32*512]: partition p, chunk j holds
    # W.T[j*128 + p, :] (contiguous per-partition runs)
    wtp = nc.dram_tensor("wtp", [128, 32 * D_SH], f8, kind="ExternalInput")
    qcol = nc.dram_tensor("qcol", [128, 32], f8, kind="ExternalInput")
    brow = nc.dram_tensor("brow", [1, D_SH], f32, kind="ExternalInput")
    ones8 = nc.dram_tensor("ones8", [128, 1], f8, kind="ExternalInput")
    neg2 = nc.dram_tensor("neg2", [128, 1], f8, kind="ExternalInput")
    ones32 = nc.dram_tensor("ones32", [128, 1], f32, kind="ExternalInput")
    ident = nc.dram_tensor("ident", [4, 4], f32, kind="ExternalInput")
    outp = nc.dram_tensor("outp", [33, 512], f32, kind="ExternalOutput")

    with tile.TileContext(nc) as tc:
        with (
            tc.tile_pool(name="const", bufs=1) as cpool,
            tc.tile_pool(name="mem", bufs=8) as mpool,
            tc.tile_pool(name="dts", bufs=3) as dpool,
            tc.tile_pool(name="acts", bufs=2) as apool,
            tc.tile_pool(name="cp", bufs=4) as cppool,
            tc.tile_pool(name="small", bufs=1) as spool,
            tc.tile_pool(name="psg", bufs=1, space="PSUM") as ppg,
            tc.tile_pool(name="psm", bufs=6, space="PSUM") as ppm,
        ):
            # gate weights first on the sync ring (mem stream queues
            # behind); small tensors on the scalar ring.
            wt_sb = cpool.tile([128, 32 * D_SH], f8, tag="wt")
            for h in range(4):
                sl = slice(h * 8 * D_SH, (h + 1) * 8 * D_SH)
                nc.sync.dma_start(out=wt_sb[:, sl], in_=wtp[:, sl])
            qc_sb = spool.tile([128, 32], f8, tag="qc")
            nc.scalar.dma_start(out=qc_sb[:], in_=qcol[:])
            b_sb = spool.tile([1, D_SH], f32, tag="b")
            nc.scalar.dma_start(out=b_sb[:], in_=brow[:])
            ones_sb = spool.tile([128, 1], f8, tag="ones")
            nc.scalar.dma_start(out=ones_sb[:], in_=ones8[:])
            neg2_sb = spool.tile([128, 1], f8, tag="neg2")
            nc.scalar.dma_start(out=neg2_sb[:], in_=neg2[:])
            ones32_sb = spool.tile([128, 1], f32, tag="ones32")
            nc.scalar.dma_start(out=ones32_sb[:], in_=ones32[:])
            id_sb = spool.tile([4, 4], f32, tag="id")
            nc.scalar.dma_start(out=id_sb[:], in_=ident[:])
            # preload Sigmoid+Abs activation tables off the critical path
            dum = spool.tile([1, 16], f32, tag="dum")
            nc.scalar.activation(dum[:], b_sb[0:1, 0:16], AF.Sigmoid)
            nc.scalar.activation(dum[:], b_sb[0:1, 0:16], AF.Abs)

            # ---- gate: z[1, 512] = sum_j qcol[:, j]^T @ wt chunk j ----
            zps = ppg.tile([1, D_SH], f32, tag="z")
            for j in range(32):
                nc.tensor.matmul(
                    zps[:],
                    qc_sb[:, j : j + 1],
                    wt_sb[:, j * D_SH : (j + 1) * D_SH],
                    start=(j == 0),
                    stop=(j == 31),
                    skip_group_check=True,
                )
            zrow = spool.tile([1, D_SH], f32, tag="zrow")
            nc.vector.tensor_tensor(zrow[:], zps[:], b_sb[:], A.add)
            grow = spool.tile([1, D_SH], f32, tag="grow")
            nc.scalar.activation(grow[:], zrow[:], AF.Sigmoid)
            # scatter the row to 4 partitions, then PE-transpose to columns
            g4 = spool.tile([4, 128], f32, tag="g4")
            nc.scalar.dma_start(
                out=g4[:], in_=grow[:].rearrange("p (a b) -> p a b", a=4)
            )
            tps = ppg.tile([128, DC], f32, tag="tp")
            nc.tensor.transpose(tps[:], g4[:], id_sb[:])
            gpos = spool.tile([128, DC], f32, tag="gpos")
            nc.vector.tensor_copy(gpos[:], tps[:])
            negg = spool.tile([128, DC], f32, tag="negg")
            nc.vector.tensor_scalar(negg[:], tps[:], -1.0, None, A.mult)
            # per-d-chunk gate sums -> psum row (rides out via outp[32])
            gs = zps[0:1, 0:DC]
            nc.tensor.matmul(
                gs, ones32_sb[:], gpos[:], start=True, stop=True,
                skip_group_check=True,
            )
            gs_sb = spool.tile([1, DC], f32, tag="gs")
            nc.vector.tensor_copy(gs_sb[:], gs)
            nc.gpsimd.dma_start(out=outp[32:33, 0:DC], in_=gs_sb[:])

            # ---- main loop ----
            for k in range(NK):
                bank0 = ppm.tile([128, 512], f32, tag="bank")
                bank1 = ppm.tile([128, 512], f32, tag="bank")
                banks = [bank0, bank1]
                # per-group pass budget: DVE chunks cost 2 (m + min),
                # ACT chunks cost 1 (abs)
                total_passes = sum(
                    1 if (k, c) in ACT_TILES else 2 for c in range(DC)
                )
                seen = [0] * NG
                mts = []
                # phase 1: DMA + gate-independent m-term matmuls
                for c in range(DC):
                    mt = mpool.tile([128, NT], f8, tag="m")
                    nc.sync.dma_start(
                        out=mt[:],
                        in_=memT[c * 128 : (c + 1) * 128, k * NT : (k + 1) * NT],
                    )
                    mts.append(mt)
                    if (k, c) not in ACT_TILES:
                        for j in range(NG):
                            nc.tensor.matmul(
                                banks[j // 4][32 * (j % 4) : 32 * (j % 4) + 1, :],
                                ones_sb[:],
                                mt[:, j * 512 : (j + 1) * 512],
                                start=(seen[j] == 0),
                                stop=(seen[j] == total_passes - 1),
                                tile_position=(0, 32 * (j % 4)),
                                skip_group_check=True,
                            )
                            seen[j] += 1
                # phase 2: gate-dependent passes
                for c in range(DC):
                    mt = mts[c]
                    if (k, c) in ACT_TILES:
                        at = apool.tile([128, NT], f8, tag="a")
                        nc.scalar.activation(
                            at[:], mt[:], AF.Abs, bias=negg[:, c : c + 1]
                        )
                        src, stat = at, ones_sb
                    else:
                        dt = dpool.tile([128, NT], f8, tag="d")
                        nc.vector.tensor_scalar(
                            dt[:], mt[:], gpos[:, c : c + 1], 0.0,
                            A.subtract, A.min,
                        )
                        src, stat = dt, neg2_sb
                    for j in range(NG):
                        nc.tensor.matmul(
                            banks[j // 4][32 * (j % 4) : 32 * (j % 4) + 1, :],
                            stat[:],
                            src[:, j * 512 : (j + 1) * 512],
                            start=(seen[j] == 0),
                            stop=(seen[j] == total_passes - 1),
                            tile_position=(0, 32 * (j % 4)),
                            skip_group_check=True,
                        )
                        seen[j] += 1
                for h in range(2):
                    cp = cppool.tile([128, 512], f32, tag="cp")
                    if h == 0 or k in (0, 2):
                        nc.vector.tensor_copy(cp[:], banks[h][:])
                    else:
                        nc.scalar.activation(cp[:], banks[h][:], AF.Copy)
                    nc.gpsimd.dma_start(
                        out=outp[8 * k + 4 * h : 8 * k + 4 * h + 4, :],
                        in_=cp[0:128:32, :],
                    )

    nc.compile()
    return nc


def _get_nc():
    if "nc" not in _CACHE:
        _CACHE["nc"] = _build()
    return _CACHE["nc"]


def kernel(query, W, b, memory, _trace=False, _return_raw=False):
    f8 = ml_dtypes.float8_e3m4
    query = np.asarray(query, dtype=np.float32)
    W = np.asarray(W, dtype=np.float32)
    b = np.asarray(b, dtype=np.float32)
    memory = np.asarray(memory, dtype=np.float32)

    mem8T = np.ascontiguousarray(memory.astype(f8).T)       # [D, N] fp8
    W8 = W.astype(f8)
    q8 = np.ascontiguousarray(query.reshape(32, 128).astype(f8).T)  # [128, 32]
    ones = np.ones((128, 1), dtype=f8)
    neg2 = np.full((128, 1), -2.0, dtype=f8)
    ones32 = np.ones((128, 1), dtype=np.float32)
    ident = np.eye(4, dtype=np.float32)

    in_maps = []
    for c in range(N_CORES):
        sl = slice(c * D_SH, (c + 1) * D_SH)
        # wtp[p, j*512 + n] = W.T[j*128 + p, n] = W8[sl][n, j*128+p]
        wsh = W8[sl, :]                       # [512, 4096]
        wtp = np.ascontiguousarray(
            wsh.T.reshape(32, 128, D_SH).transpose(1, 0, 2).reshape(128, -1)
        )
        in_maps.append(
            {
                "memT": np.ascontiguousarray(mem8T[sl, :]),
                "wtp": wtp,
                "qcol": q8,
                "brow": np.ascontiguousarray(b[sl].reshape(1, D_SH)),
                "ones8": ones,
                "neg2": neg2,
                "ones32": ones32,
                "ident": ident,
            }
        )

    nc = _get_nc()
    res = run_bass_kernel_spmd(
        nc, in_maps, list(range(N_CORES)), trace=_trace
    )

    total = np.zeros(N, dtype=np.float64)
    for c in range(N_CORES):
        out = res.results[c]["outp"]
        gsum = out[32, 0:DC].astype(np.float64)   # sum of g per d-chunk
        rows = out[0:32].reshape(NK, NG, 512)
        corr = np.array(
            [sum(gsum[ci] for ci in DVE_SETS[k]) for k in range(NK)]
        )
        total += (rows - corr[:, None, None]).reshape(N)
    sims = (1.0 - total / D).astype(np.float32)
    mask = sims >= THRESHOLD
    if _return_raw:
        return (sims, mask), res
    return sims, mask


# revision 10
# speedup vs baseline: 2.0147x; 1.0871x over previous
"""Trainium2 Bass kernel for nn_BinaryMemory (retrieval_knn).

reference:
    gated = sigmoid(query @ W.T + b)                      # [1, D], D=4096
    sims  = 1 - mean(|memory - gated|, axis=-1)           # [N],   N=16384
    mask  = sims >= 0.8

Sharding (8 cores, no collectives): shard the D axis; core c owns
d-chunk [c*512, (c+1)*512). All bulk tensors stream as fp8_e3m4 (1 byte,
4 mantissa bits; operands live in (0,1) or N(0,1), quantization puts
~1e-2 relative on sims vs the 2e-2 budget). Per-core HBM ~10.1 MB; one
HWDGE ring sustains ~320 GB/s (a second ring adds nothing - per-core
HBM limit), so DMA floors the kernel at ~33 us + init.

Layout is d-on-partitions (memory shard transposed host-side to
[512 d, 16384 n]) so the gate value g[d] is a per-partition scalar.
The elementwise work splits |m-g| = (m-g) - 2*min(m-g, 0):
  - DVE tiles: ONE stock pass, tensor_scalar(op0=subtract scalar1=g,
    op1=min scalar2=0) -> min-term (fp8 in/out hits the 2x port mode,
    ~2.26 us per [128,4096]; a fused sub+abs is not expressible: the
    ISA rejects abs_max/bitwise op1 pairings and accum_out forces 1x).
  - ScalarE tiles: one Abs activation with per-partition bias -g
    (|m-g| directly, ~3.6 us/tile, dtype-agnostic).
The n-reduction runs on the otherwise-idle PE: per 512-column group a
psum row accumulates ones^T @ m (issued as soon as the tile lands - no
gate dependency) plus (-2*ones)^T @ minterm for DVE tiles, or
ones^T @ |m-g| for ScalarE tiles. The spurious +sum(g) from the m-term
is cancelled on the host via per-d-chunk gate sums (psum row 32 of the
output). Each psum bank holds 4 group-rows at quadrant offsets
{0,32,64,96} (tile_position); one [128,512] psum->SBUF copy drains 4
groups and a partition-strided DMA writes the rows to DRAM on the
otherwise-idle SWDGE (gpsimd) ring.

Gate: host pre-packs W-shard^T as [128, 32*512] fp8, DMA'd first on the
sync ring in 4 chunk-tiles so matmuls start as chunks land. q rides as
32 stationary columns; matmul j targets quadrant strip 32*(j%4)
(tile_position) so the PE's reorder window pulls each LDWEIGHTS ahead
of the in-flight MATMUL on the neighbouring strip - the serial LDW+MM
turnaround (~300 ns) drops to pipelined rate. The 4 partial z-rows are
copied out of psum, re-scattered to per-partition columns by a
transposed-AP SBUF->SBUF DMA, summed on DVE ([128,4,4] X-reduce),
biased with b, sigmoided on [128,4], and negated - no PE transpose, no
[1,512] row ops. Activation tables preload via dummy ops at t0, and
the small constants ride in two packed dram tensors (2 descriptors so
the scalar ring frees up early).
"""
import sys

sys.path.insert(0, "/opt/trn_rl_repo")

import numpy as np
import ml_dtypes

import concourse.bacc as bacc
import concourse.mybir as mybir
import concourse.tile as tile
from concourse.bass_utils import run_bass_kernel_spmd

N_CORES = 8
D = 4096
N = 16384
D_SH = D // N_CORES            # 512 dims per core
DC = D_SH // 128               # 4 d-chunks (partition blocks)
NT = 4096                      # n per tile
NK = N // NT                   # 4 n-chunks
NG = NT // 512                 # 8 psum groups per tile
THRESHOLD = 0.8
# tiles (k, c) computed on ScalarE via Abs; the rest on DVE via sub+min
ACT_TILES = {(0, 2), (1, 1), (1, 3), (2, 0), (3, 1), (3, 3)}
DVE_SETS = [
    [c for c in range(DC) if (k, c) not in ACT_TILES] for k in range(NK)
]

_CACHE = {}


def _build():
    f32 = mybir.dt.float32
    f8 = mybir.dt.float8e3
    A = mybir.AluOpType
    AF = mybir.ActivationFunctionType
    nc = bacc.Bacc(
        "TRN2", target_bir_lowering=False, debug=False, num_devices=N_CORES
    )

    memT = nc.dram_tensor("memT", [D_SH, N], f8, kind="ExternalInput")
    # W shard, host-packed: partition p, chunk j holds W.T[j*128 + p, :]
    wtp = nc.dram_tensor("wtp", [128, 32 * D_SH], f8, kind="ExternalInput")
    # packed constants: cols 0:32 qcol, 32 ones, 33 neg2
    c8 = nc.dram_tensor("c8", [128, 34], f8, kind="ExternalInput")
    # packed f32 constants: cols 0:4 b columns, 4 ones, 5:9 eye(4)
    c32 = nc.dram_tensor("c32", [128, 9], f32, kind="ExternalInput")
    outp = nc.dram_tensor("outp", [33, 512], f32, kind="ExternalOutput")

    with tile.TileContext(nc) as tc:
        with (
            tc.tile_pool(name="wts", bufs=1) as wpool,
            tc.tile_pool(name="mem", bufs=10) as mpool,
            tc.tile_pool(name="dts", bufs=3) as dpool,
            tc.tile_pool(name="acts", bufs=2) as apool,
            tc.tile_pool(name="cp", bufs=6) as cppool,
            tc.tile_pool(name="small", bufs=1) as spool,
            tc.tile_pool(name="psg", bufs=1, space="PSUM") as ppg,
            tc.tile_pool(name="psm", bufs=7, space="PSUM") as ppm,
        ):
            # gate weights first on the sync ring (mem stream queues
            # behind); 2 packed constant tensors on the scalar ring.
            wts = []
            for h in range(4):
                wt_sb = wpool.tile([128, 8 * D_SH], f8, tag=f"wt{h}")
                nc.sync.dma_start(
                    out=wt_sb[:],
                    in_=wtp[:, h * 8 * D_SH : (h + 1) * 8 * D_SH],
                )
                wts.append(wt_sb)
            c8_sb = spool.tile([128, 34], f8, tag="c8")
            nc.scalar.dma_start(out=c8_sb[:], in_=c8[:])
            c32_sb = spool.tile([128, 9], f32, tag="c32")
            nc.scalar.dma_start(out=c32_sb[:], in_=c32[:])
            qc_sb = c8_sb[:, 0:32]
            ones_sb = c8_sb[:, 32:33]
            neg2_sb = c8_sb[:, 33:34]
            b4 = c32_sb[:, 0:4]
            ones32_sb = c32_sb[:, 4:5]
            id4 = c32_sb[0:4, 5:9]
            # preload Sigmoid+Abs activation tables off the critical path
            dum = spool.tile([1, 4], f32, tag="dum")
            nc.scalar.activation(dum[:], c32_sb[0:1, 0:4], AF.Sigmoid)
            nc.scalar.activation(dum[:], c32_sb[0:1, 0:4], AF.Abs)

            # ---- gate: 4 quadrant strips accumulate partial z rows ----
            zps = ppg.tile([128, D_SH], f32, tag="z")
            for j in range(32):
                r = j % 4
                nc.tensor.matmul(
                    zps[32 * r : 32 * r + 1, :],
                    qc_sb[:, j : j + 1],
                    wts[j // 8][:, (j % 8) * D_SH : (j % 8 + 1) * D_SH],
                    start=(j < 4),
                    stop=(j >= 28),
                    tile_position=(0, 32 * r),
                    skip_group_check=True,
                )
            zcp = spool.tile([128, D_SH], f32, tag="zcp")
            nc.scalar.activation(zcp[:], zps[:], AF.Copy)
            # pack the 4 strided strip rows into 4 partitions, transpose
            # each output-chunk on the PE, reduce strips from psum on DVE
            zcp4 = spool.tile([4, D_SH], f32, tag="zcp4")
            nc.scalar.dma_start(out=zcp4[:], in_=zcp[0:128:32, :])
            tps = zps[:, 0:16]
            for c in range(DC):
                nc.tensor.transpose(
                    tps[:, 4 * c : 4 * (c + 1)],
                    zcp4[:, c * 128 : (c + 1) * 128],
                    id4,
                )
            zred = spool.tile([128, DC], f32, tag="zred")
            nc.vector.tensor_reduce(
                out=zred[:],
                in_=tps.rearrange("p (c r) -> p c r", r=4),
                axis=mybir.AxisListType.X,
                op=A.add,
            )
            zb = spool.tile([128, DC], f32, tag="zb")
            nc.vector.tensor_tensor(zb[:], zred[:], b4, A.add)
            gpos = spool.tile([128, DC], f32, tag="gpos")
            nc.scalar.activation(gpos[:], zb[:], AF.Sigmoid)
            negg = spool.tile([128, DC], f32, tag="negg")
            nc.vector.tensor_scalar(negg[:], gpos[:], -1.0, None, A.mult)
            # per-d-chunk gate sums for the host-side m-term correction;
            # reuses the zps bank after zcp drains it (WAR tracked)
            gs = zps[0:1, 500:504]
            nc.tensor.matmul(
                gs, ones32_sb, gpos[:], start=True, stop=True,
                skip_group_check=True,
            )
            gs_sb = spool.tile([1, DC], f32, tag="gs")
            nc.vector.tensor_copy(gs_sb[:], gs)
            nc.gpsimd.dma_start(out=outp[32:33, 0:DC], in_=gs_sb[:])

            # ---- main loop ----
            for k in range(NK):
                bank0 = ppm.tile([128, 512], f32, tag="bank")
                bank1 = ppm.tile([128, 512], f32, tag="bank")
                banks = [bank0, bank1]
                total_passes = sum(
                    1 if (k, c) in ACT_TILES else 2 for c in range(DC)
                )
                seen = [0] * NG
                mts = []
                # phase 1: DMA + gate-independent m-term matmuls
                for c in range(DC):
                    mt = mpool.tile([128, NT], f8, tag="m")
                    nc.sync.dma_start(
                        out=mt[:],
                        in_=memT[c * 128 : (c + 1) * 128, k * NT : (k + 1) * NT],
                    )
                    mts.append(mt)
                    if (k, c) not in ACT_TILES:
                        for j in range(NG):
                            nc.tensor.matmul(
                                banks[j // 4][32 * (j % 4) : 32 * (j % 4) + 1, :],
                                ones_sb,
                                mt[:, j * 512 : (j + 1) * 512],
                                start=(seen[j] == 0),
                                stop=(seen[j] == total_passes - 1),
                                tile_position=(0, 32 * (j % 4)),
                                skip_group_check=True,
                            )
                            seen[j] += 1
                # phase 2: gate-dependent passes
                for c in range(DC):
                    mt = mts[c]
                    if (k, c) in ACT_TILES:
                        at = apool.tile([128, NT], f8, tag="a")
                        nc.scalar.activation(
                            at[:], mt[:], AF.Abs, bias=negg[:, c : c + 1]
                        )
                        src, stat = at, ones_sb
                    else:
                        dt = dpool.tile([128, NT], f8, tag="d")
                        nc.vector.tensor_scalar(
                            dt[:], mt[:], gpos[:, c : c + 1], 0.0,
                            A.subtract, A.min,
                        )
                        src, stat = dt, neg2_sb
                    for j in range(NG):
                        nc.tensor.matmul(
                            banks[j // 4][32 * (j % 4) : 32 * (j % 4) + 1, :],
                            stat,
                            src[:, j * 512 : (j + 1) * 512],
                            start=(seen[j] == 0),
                            stop=(seen[j] == total_passes - 1),
                            tile_position=(0, 32 * (j % 4)),
                            skip_group_check=True,
                        )
                        seen[j] += 1
                for h in range(2):
                    cp = cppool.tile([128, 512], f32, tag="cp")
                    if h == 0:
                        nc.vector.tensor_copy(cp[:], banks[h][:])
                    else:
                        nc.scalar.activation(cp[:], banks[h][:], AF.Copy)
                    nc.gpsimd.dma_start(
                        out=outp[8 * k + 4 * h : 8 * k + 4 * h + 4, :],
                        in_=cp[0:128:32, :],
                    )

    nc.compile()
    return nc


def _get_nc():
    if "nc" not in _CACHE:
        _CACHE["nc"] = _build()
    return _CACHE["nc"]


def kernel(query, W, b, memory, _trace=False, _return_raw=False):
    f8 = ml_dtypes.float8_e3m4
    query = np.asarray(query, dtype=np.float32)
    W = np.asarray(W, dtype=np.float32)
    b = np.asarray(b, dtype=np.float32)
    memory = np.asarray(memory, dtype=np.float32)

    mem8T = np.ascontiguousarray(memory.astype(f8).T)       # [D, N] fp8
    W8 = W.astype(f8)
    q8 = query.reshape(32, 128).astype(f8).T                # [128, 32]
    c8 = np.empty((128, 34), dtype=f8)
    c8[:, 0:32] = q8
    c8[:, 32] = f8(1.0)
    c8[:, 33] = f8(-2.0)

    in_maps = []
    for c in range(N_CORES):
        sl = slice(c * D_SH, (c + 1) * D_SH)
        # wtp[p, j*512 + n] = W.T[j*128 + p, n] = W8[sl][n, j*128+p]
        wsh = W8[sl, :]                       # [512, 4096]
        wtp = np.ascontiguousarray(
            wsh.T.reshape(32, 128, D_SH).transpose(1, 0, 2).reshape(128, -1)
        )
        c32 = np.zeros((128, 9), dtype=np.float32)
        c32[:, 0:4] = b[sl].reshape(4, 128).T
        c32[:, 4] = 1.0
        c32[0:4, 5:9] = np.eye(4, dtype=np.float32)
        in_maps.append(
            {
                "memT": np.ascontiguousarray(mem8T[sl, :]),
                "wtp": wtp,
                "c8": c8,
                "c32": c32,
            }
        )

    nc = _get_nc()
    res = run_bass_kernel_spmd(
        nc, in_maps, list(range(N_CORES)), trace=_trace
    )

    total = np.zeros(N, dtype=np.float64)
    for c in range(N_CORES):
        out = res.results[c]["outp"]
        gsum = out[32, 0:DC].astype(np.float64)   # sum of g per d-chunk
        rows = out[0:32].reshape(NK, NG, 512)
        corr = np.array(
            [sum(gsum[ci] for ci in DVE_SETS[k]) for k in range(NK)]
        )
        total += (rows - corr[:, None, None]).reshape(N)
    sims = (1.0 - total / D).astype(np.float32)
    mask = sims >= THRESHOLD
    if _return_raw:
        return (sims, mask), res
    return sims, mask
